# revision 3
# baseline (speedup 1.0000x reference)
"""Trainium2 kernel for SparseLinear + bias + SELU (nn_AEEncoder).

Reference computation:
    y[b, o] = selu( sum_{e: out_idx[e]==o} weight[e] * x[b, in_idx[e]] + bias[o] )
with B=512, IN_F=20000, OUT_F=1000, NNZ=500000.

Strategy (v2: int8 wire format)
-------------------------------
The edge list is densified on the host into W[IN_F, OUT_F]; the device runs
a dense matmul + SELU. Sharding is a 2 (batch) x 4 (out-col) grid over the
8 cores, no collectives; each core computes a [256, 250] output block with
K = 20096 (IN_F padded, one row carries the bias fold).

The baseline shipped x^T / W as bf16 (20.3 MB/core) and was DMA-bound at
~70us (per-core DMA ~330 GB/s). v2 ships both operands as *int8* (symmetric
quantization, clip at 4 sigma -> rel err ~1.5e-2 < 2e-2 budget), halving
DMA to ~10.3 MB/core. On-chip, DVE + ACT upcast int8 -> bf16 (int8 values
are exact in bf16; DVE runs the cast at 2 elem/cycle/lane), and the PE runs
the same bf16 matmul stream (314 matmuls of N=250 -> ~34us, now the binding
engine). The dequant scale (dx*dw) rides in as a [128, 2] input and is
folded into the SELU epilogue's activation scales.

Pipeline per k-group: DMA int8 (x on sync queue, w on a second queue,
group boundaries staggered so inter-DMA gaps hide under the other queue's
transfer) -> cast to bf16 (DVE takes all of x plus a subset of w k-groups,
ACT the rest) -> PE accumulating matmuls. Dummy matmuls on a zeroed tile at
t=0 warm the PE out of its low p-state during DMA startup. The last TAIL_M
k-tiles run m-major so the m=0 epilogue overlaps m=1's final matmuls.
"""

import math

import numpy as np
import ml_dtypes

import concourse.bass as bass
import concourse.mybir as mybir
import concourse.tile as tile
from concourse import bacc
from concourse.bass_utils import run_bass_kernel_spmd

B, IN_F, OUT_F = 512, 20000, 1000
NCORES = 8
BS, OS = 2, 4          # batch split x out-column split
BSH = B // BS          # 256 batch rows per core
OSH = OUT_F // OS      # 250 output columns per core
WPAD = 256             # w k-tile padded to 256 cols (DMA/cast alignment)
KPAD = 20096           # padded contraction dim: 157 k-tiles of 128
KT = KPAD // 128       # 157 k-tiles (row IN_F==20000 carries the bias)
MT = BSH // 128        # 2 M-tiles per core

# k-groups for the two DMA streams; both sum to KT. Staggered boundaries so
# the two queues' inter-DMA gaps do not line up.
GROUPS_X = [4, 10, 18, 22, 22, 22, 22, 18, 12, 7]
GROUPS_W = [6, 12, 20, 22, 22, 22, 22, 16, 10, 5]
DVE_W_GROUPS = {1, 4, 7}   # w k-groups cast by DVE (rest on ACT)
ACT_CAST_SPLIT = 11        # max k-tiles per ACT cast instruction
W_QUEUE = "scalar"         # engine issuing the w DMA stream
TAIL_M = 16            # last k-tiles emitted m-major (epilogue overlap)
NCHUNK = 2             # epilogue column chunks
WARMUP = 14            # dummy matmuls (N=256) to ramp the PE p-state
CLIP_SIGMA = 4.0

SELU_SCALE = 1.0507009873554805
SELU_ALPHA = 1.6732632423543772
ONES_ROW_VAL = 32.0    # exact-in-int8 value for the bias fold row of x

_compiled = None


def _build():
    lam_al = SELU_SCALE * SELU_ALPHA
    nc = bacc.Bacc("TRN2", target_bir_lowering=False, debug=False,
                   num_devices=NCORES)
    xq_d = nc.dram_tensor("xq", [128, KT, BSH], mybir.dt.int8,
                          kind="ExternalInput")
    wq_d = nc.dram_tensor("wq", [128, KT, WPAD], mybir.dt.int8,
                          kind="ExternalInput")
    # dequant scales: col 0 = S = dx*dw, col 1 = lam*S
    sc_d = nc.dram_tensor("sc", [128, 2], mybir.dt.float32,
                          kind="ExternalInput")
    out_d = nc.dram_tensor("out", [BSH, OSH], mybir.dt.bfloat16,
                           kind="ExternalOutput")

    GXMAX = max(GROUPS_X)
    GWMAX = max(GROUPS_W)
    wq_eng = {"scalar": nc.scalar, "sync": nc.sync,
              "gpsimd": nc.gpsimd, "vector": nc.vector}[W_QUEUE]

    with tile.TileContext(nc) as tc:
        with (
            tc.tile_pool(name="sb", bufs=1) as sb,
            tc.tile_pool(name="ps", bufs=1, space="PSUM") as ps,
        ):
            # PE warmup: dummy matmuls on a zeroed tile into a scratch bank
            zz = sb.tile([128, 256], mybir.dt.bfloat16)
            scr = ps.tile([128, 512], mybir.dt.float32, name="scr", tag="scr")
            nc.vector.memset(zz[:], 0.0)
            for _ in range(WARMUP):
                nc.tensor.matmul(scr[:, 0:256], zz[:, 0:128], zz[:],
                                 start=True, stop=True)

            # staging rings (int8) + full bf16 operand streams
            xst = [sb.tile([128, GXMAX, BSH], mybir.dt.int8,
                           name=f"xst{i}", tag=f"xst{i}") for i in range(2)]
            wst = [sb.tile([128, GWMAX, WPAD], mybir.dt.int8,
                           name=f"wst{i}", tag=f"wst{i}") for i in range(2)]
            xbf = sb.tile([128, KT, BSH], mybir.dt.bfloat16)
            wbf = sb.tile([128, KT, WPAD], mybir.dt.bfloat16)

            # scale vector + exp bias constant
            svec = sb.tile([128, 2], mybir.dt.float32)
            nc.sync.dma_start(svec[:], sc_d[:])
            lnb = sb.tile([128, 1], mybir.dt.float32)
            nc.vector.memset(lnb[:], math.log(lam_al))

            # x stream: DMA int8 on sync queue, cast on DVE (contiguous)
            g0 = 0
            for g, gsz in enumerate(GROUPS_X):
                g1 = g0 + gsz
                st = xst[g % 2]
                nc.sync.dma_start(st[:, 0:gsz, :], xq_d[:, g0:g1, :])
                nc.vector.tensor_scalar_mul(
                    xbf[:, g0:g1, :], st[:, 0:gsz, :], 1.0)
                g0 = g1

            # w stream: DMA int8; per-group cast on DVE or ACT (contiguous)
            g0 = 0
            for g, gsz in enumerate(GROUPS_W):
                g1 = g0 + gsz
                st = wst[g % 2]
                wq_eng.dma_start(st[:, 0:gsz, :], wq_d[:, g0:g1, :])
                if g in DVE_W_GROUPS:
                    nc.vector.tensor_scalar_mul(
                        wbf[:, g0:g1, :], st[:, 0:gsz, :], 1.0)
                else:
                    # split for finer pipelining with the PE
                    c0 = 0
                    while c0 < gsz:
                        c1 = min(c0 + ACT_CAST_SPLIT, gsz)
                        nc.scalar.copy(wbf[:, g0 + c0:g0 + c1, :],
                                       st[:, c0:c1, :])
                        c0 = c1
                g0 = g1

            accs = [
                ps.tile([128, 512], mybir.dt.float32,
                        name=f"acc{m}", tag=f"acc{m}")
                for m in range(MT)
            ]

            def mm(k, m):
                nc.tensor.matmul(
                    accs[m][:, :OSH],
                    xbf[:, k, m * 128:(m + 1) * 128],
                    wbf[:, k, 0:OSH],
                    start=(k == 0),
                    stop=(k == KT - 1),
                )

            for k in range(KT - TAIL_M):
                for m in range(MT):
                    mm(k, m)
            for m in range(MT):
                for k in range(KT - TAIL_M, KT):
                    mm(k, m)

            # SELU epilogue with dequant: y = S*acc,
            # selu(y) = lam*relu(y) + lam*al*(exp(min(y,0)) - 1)
            CW = OSH // NCHUNK
            for m in range(MT):
                rl = sb.tile([128, OSH], mybir.dt.float32,
                             name=f"rl{m}", tag=f"rl{m}")
                mn = sb.tile([128, OSH], mybir.dt.float32,
                             name=f"mn{m}", tag=f"mn{m}")
                ex = sb.tile([128, OSH], mybir.dt.float32,
                             name=f"ex{m}", tag=f"ex{m}")
                oo = sb.tile([128, OSH], mybir.dt.bfloat16,
                             name=f"oo{m}", tag=f"oo{m}")
                for h in range(NCHUNK):
                    cs = slice(h * CW, (h + 1) * CW if h < NCHUNK - 1
                               else OSH)
                    # rl = relu(lam*S*acc)
                    nc.scalar.activation(rl[:, cs], accs[m][:, cs],
                                         mybir.ActivationFunctionType.Relu,
                                         scale=svec[:, 1:2])
                    # mn = min(S*acc, 0)
                    nc.vector.tensor_scalar(mn[:, cs], accs[m][:, cs],
                                            svec[:, 0:1], 0.0,
                                            mybir.AluOpType.mult,
                                            mybir.AluOpType.min)
                    # ex = lam*al*exp(mn)
                    nc.scalar.activation(ex[:, cs], mn[:, cs],
                                         mybir.ActivationFunctionType.Exp,
                                         bias=lnb[:])
                    nc.vector.scalar_tensor_tensor(
                        oo[:, cs], ex[:, cs], -lam_al, rl[:, cs],
                        mybir.AluOpType.add, mybir.AluOpType.add)
                nc.sync.dma_start(out_d[m * 128:(m + 1) * 128, :], oo[:])

    nc.compile()
    return nc


def _prepare_in_maps(x, weight, bias, out_idx, in_idx):
    x = np.asarray(x, dtype=np.float32)
    weight = np.asarray(weight, dtype=np.float32)
    bias = np.asarray(bias, dtype=np.float32)
    oi = np.asarray(out_idx).astype(np.int64)
    ii = np.asarray(in_idx).astype(np.int64)

    # densify the edge list; duplicate (i, o) pairs accumulate
    W = np.bincount(ii * OUT_F + oi, weights=weight.astype(np.float64),
                    minlength=IN_F * OUT_F).astype(np.float32)
    W = W.reshape(IN_F, OUT_F)

    dx = CLIP_SIGMA * float(x.std()) / 127.0
    dw = CLIP_SIGMA * float(weight.std()) / 127.0
    if dx == 0.0:
        dx = 1.0
    if dw == 0.0:
        dw = 1.0
    S = dx * dw

    xq = np.clip(np.rint(x / dx), -127, 127).astype(np.int8)   # [B, IN_F]
    Wq = np.clip(np.rint(W / dw), -127, 127).astype(np.int8)   # [IN_F, OUT_F]

    # padded, transposed layouts + bias fold row at k == IN_F
    xtq = np.zeros((KPAD, B), dtype=np.int8)
    xtq[:IN_F] = xq.T
    xtq[IN_F] = np.int8(ONES_ROW_VAL)
    Wpq = np.zeros((KPAD, OUT_F), dtype=np.int8)
    Wpq[:IN_F] = Wq
    bias_q = np.clip(np.rint(bias / (S * ONES_ROW_VAL)), -127, 127)
    Wpq[IN_F] = bias_q.astype(np.int8)

    sc = np.empty((128, 2), dtype=np.float32)
    sc[:, 0] = S
    sc[:, 1] = SELU_SCALE * S

    in_maps = []
    for c in range(NCORES):
        b, o = divmod(c, OS)
        xt_shard = xtq[:, b * BSH:(b + 1) * BSH]           # [KPAD, BSH]
        w_shard = Wpq[:, o * OSH:(o + 1) * OSH]            # [KPAD, OSH]
        wp = np.zeros((KT, 128, WPAD), dtype=np.int8)
        wp[:, :, :OSH] = w_shard.reshape(KT, 128, OSH)
        in_maps.append({
            # partition-major [128, KT, cols]
            "xq": np.ascontiguousarray(
                xt_shard.reshape(KT, 128, BSH).transpose(1, 0, 2)),
            "wq": np.ascontiguousarray(wp.transpose(1, 0, 2)),
            "sc": sc,
        })
    return in_maps


def _assemble(results):
    y = np.empty((B, OUT_F), dtype=np.float32)
    for c in range(NCORES):
        b, o = divmod(c, OS)
        y[b * BSH:(b + 1) * BSH, o * OSH:(o + 1) * OSH] = \
            np.asarray(results[c]["out"]).astype(np.float32)
    return y


def get_compiled():
    global _compiled
    if _compiled is None:
        _compiled = _build()
    return _compiled


def kernel(x, weight, bias, out_idx, in_idx):
    in_maps = _prepare_in_maps(x, weight, bias, out_idx, in_idx)
    nc = get_compiled()
    last_err = None
    for _attempt in range(3):  # retry transient device/runtime hiccups
        try:
            res = run_bass_kernel_spmd(nc, in_maps,
                                       core_ids=list(range(NCORES)))
            return _assemble(res.results)
        except Exception as e:  # noqa: BLE001
            last_err = e
    raise last_err


# revision 5
# speedup vs baseline: 1.1356x; 1.1356x over previous
"""Trainium2 kernel for SparseLinear + bias + SELU (nn_AEEncoder).

Reference computation:
    y[b, o] = selu( sum_{e: out_idx[e]==o} weight[e] * x[b, in_idx[e]] + bias[o] )
with B=512, IN_F=20000, OUT_F=1000, NNZ=500000.

Strategy (v2: int8 wire format)
-------------------------------
The edge list is densified on the host into W[IN_F, OUT_F]; the device runs
a dense matmul + SELU. Sharding is a 2 (batch) x 4 (out-col) grid over the
8 cores, no collectives; each core computes a [256, 250] output block with
K = 20096 (IN_F padded, one row carries the bias fold).

The baseline shipped x^T / W as bf16 (20.3 MB/core) and was DMA-bound at
~70us (per-core DMA ~330 GB/s). v2 ships both operands as *int8* (symmetric
quantization, clip at 4 sigma -> rel err ~1.5e-2 < 2e-2 budget), halving
DMA to ~10.3 MB/core. On-chip, DVE + ACT upcast int8 -> bf16 (int8 values
are exact in bf16; DVE runs the cast at 2 elem/cycle/lane), and the PE runs
the same bf16 matmul stream (314 matmuls of N=250 -> ~34us, now the binding
engine). The dequant scale (dx*dw) rides in as a [128, 2] input and is
folded into the SELU epilogue's activation scales.

Pipeline per k-group: DMA int8 (x on sync queue, w on a second queue,
group boundaries staggered so inter-DMA gaps hide under the other queue's
transfer) -> cast to bf16 (DVE takes all of x plus a subset of w k-groups,
ACT the rest) -> PE accumulating matmuls. Dummy matmuls on a zeroed tile at
t=0 warm the PE out of its low p-state during DMA startup. The last TAIL_M
k-tiles run m-major so the m=0 epilogue overlaps m=1's final matmuls.
"""

import math

import numpy as np
import ml_dtypes

import concourse.bass as bass
import concourse.mybir as mybir
import concourse.tile as tile
from concourse import bacc
from concourse.bass_utils import run_bass_kernel_spmd

B, IN_F, OUT_F = 512, 20000, 1000
NCORES = 8
BS, OS = 2, 4          # batch split x out-column split
BSH = B // BS          # 256 batch rows per core
OSH = OUT_F // OS      # 250 output columns per core
WPAD = 256             # w k-tile padded to 256 cols (DMA/cast alignment)
KPAD = 20096           # padded contraction dim: 157 k-tiles of 128
KT = KPAD // 128       # 157 k-tiles (row IN_F==20000 carries the bias)
MT = BSH // 128        # 2 M-tiles per core

# k-groups for the two DMA streams; both sum to KT. Front-loaded small so
# the pipeline starts fast; staggered boundaries so the two queues'
# inter-DMA gaps do not line up.
GROUPS_X = [3, 6, 12, 18, 22, 22, 22, 22, 18, 12]
GROUPS_W = [4, 8, 14, 20, 22, 22, 22, 20, 15, 10]
W_DVE_COLS = 112           # w k-tile cols cast by DVE (rest on ACT)
ACT_CAST_SPLIT = 11        # max k-tiles per ACT cast instruction
W_QUEUE = "scalar"         # engine issuing the w DMA stream
TAIL_M = 16            # last k-tiles emitted m-major (epilogue overlap)
NCHUNK = 2             # epilogue column chunks
WARMUP = 5             # dummy matmuls (N=256) to ramp the PE p-state
CLIP_SIGMA = 4.0

SELU_SCALE = 1.0507009873554805
SELU_ALPHA = 1.6732632423543772
ONES_ROW_VAL = 32.0    # exact-in-int8 value for the bias fold row of x

_compiled = None


def _build():
    lam_al = SELU_SCALE * SELU_ALPHA
    nc = bacc.Bacc("TRN2", target_bir_lowering=False, debug=False,
                   num_devices=NCORES)
    xq_d = nc.dram_tensor("xq", [128, KT, BSH], mybir.dt.int8,
                          kind="ExternalInput")
    wq_d = nc.dram_tensor("wq", [128, KT, WPAD], mybir.dt.int8,
                          kind="ExternalInput")
    # dequant scales: col 0 = S = dx*dw, col 1 = lam*S
    sc_d = nc.dram_tensor("sc", [128, 2], mybir.dt.float32,
                          kind="ExternalInput")
    out_d = nc.dram_tensor("out", [BSH, OSH], mybir.dt.bfloat16,
                           kind="ExternalOutput")

    GXMAX = max(GROUPS_X)
    GWMAX = max(GROUPS_W)
    wq_eng = {"scalar": nc.scalar, "sync": nc.sync,
              "gpsimd": nc.gpsimd, "vector": nc.vector}[W_QUEUE]

    with tile.TileContext(nc) as tc:
        with (
            tc.tile_pool(name="sb", bufs=1) as sb,
            tc.tile_pool(name="ps", bufs=1, space="PSUM") as ps,
        ):
            # PE warmup: dummy matmuls on a zeroed tile into a scratch bank
            zz = sb.tile([128, 256], mybir.dt.bfloat16)
            scr = ps.tile([128, 512], mybir.dt.float32, name="scr", tag="scr")
            nc.vector.memset(zz[:], 0.0)
            for _ in range(WARMUP):
                nc.tensor.matmul(scr[:, 0:256], zz[:, 0:128], zz[:],
                                 start=True, stop=True)

            # staging rings (int8) + full bf16 operand streams
            xst = [sb.tile([128, GXMAX, BSH], mybir.dt.int8,
                           name=f"xst{i}", tag=f"xst{i}") for i in range(2)]
            wst = [sb.tile([128, GWMAX, WPAD], mybir.dt.int8,
                           name=f"wst{i}", tag=f"wst{i}") for i in range(2)]
            xbf = sb.tile([128, KT, BSH], mybir.dt.bfloat16)
            wbf = sb.tile([128, KT, WPAD], mybir.dt.bfloat16)

            # scale vector + exp bias constant
            svec = sb.tile([128, 2], mybir.dt.float32)
            nc.sync.dma_start(svec[:], sc_d[:])
            lnb = sb.tile([128, 1], mybir.dt.float32)
            nc.vector.memset(lnb[:], math.log(lam_al))

            # DMA + cast emission, merged in k-order across both streams so
            # each engine's in-order queue advances with PE consumption.
            # Per x-group: DVE casts the whole group (contiguous).
            # Per w-group: DVE casts cols [0:W_DVE_COLS], ACT the rest.
            events = []   # (start_k, prio, stream, group_idx, g0, g1)
            g0 = 0
            for g, gsz in enumerate(GROUPS_W):
                events.append((g0, 0, "w", g, g0, g0 + gsz))
                g0 += gsz
            g0 = 0
            for g, gsz in enumerate(GROUPS_X):
                events.append((g0, 1, "x", g, g0, g0 + gsz))
                g0 += gsz
            events.sort()
            for _, _, stream, g, g0, g1 in events:
                gsz = g1 - g0
                if stream == "x":
                    st = xst[g % 2]
                    nc.sync.dma_start(st[:, 0:gsz, :], xq_d[:, g0:g1, :])
                    nc.vector.tensor_scalar_mul(
                        xbf[:, g0:g1, :], st[:, 0:gsz, :], 1.0)
                else:
                    st = wst[g % 2]
                    wq_eng.dma_start(st[:, 0:gsz, :], wq_d[:, g0:g1, :])
                    nc.vector.tensor_scalar_mul(
                        wbf[:, g0:g1, 0:W_DVE_COLS],
                        st[:, 0:gsz, 0:W_DVE_COLS], 1.0)
                    c0 = 0
                    while c0 < gsz:
                        c1 = min(c0 + ACT_CAST_SPLIT, gsz)
                        nc.scalar.copy(
                            wbf[:, g0 + c0:g0 + c1, W_DVE_COLS:WPAD],
                            st[:, c0:c1, W_DVE_COLS:WPAD])
                        c0 = c1

            accs = [
                ps.tile([128, 512], mybir.dt.float32,
                        name=f"acc{m}", tag=f"acc{m}")
                for m in range(MT)
            ]

            def mm(k, m):
                nc.tensor.matmul(
                    accs[m][:, :OSH],
                    xbf[:, k, m * 128:(m + 1) * 128],
                    wbf[:, k, 0:OSH],
                    start=(k == 0),
                    stop=(k == KT - 1),
                )

            for k in range(KT - TAIL_M):
                for m in range(MT):
                    mm(k, m)
            for m in range(MT):
                for k in range(KT - TAIL_M, KT):
                    mm(k, m)

            # SELU epilogue with dequant: y = S*acc,
            # selu(y) = lam*relu(y) + lam*al*(exp(min(y,0)) - 1)
            CW = OSH // NCHUNK
            for m in range(MT):
                rl = sb.tile([128, OSH], mybir.dt.float32,
                             name=f"rl{m}", tag=f"rl{m}")
                mn = sb.tile([128, OSH], mybir.dt.float32,
                             name=f"mn{m}", tag=f"mn{m}")
                ex = sb.tile([128, OSH], mybir.dt.float32,
                             name=f"ex{m}", tag=f"ex{m}")
                oo = sb.tile([128, OSH], mybir.dt.bfloat16,
                             name=f"oo{m}", tag=f"oo{m}")
                for h in range(NCHUNK):
                    cs = slice(h * CW, (h + 1) * CW if h < NCHUNK - 1
                               else OSH)
                    # rl = relu(lam*S*acc)
                    nc.scalar.activation(rl[:, cs], accs[m][:, cs],
                                         mybir.ActivationFunctionType.Relu,
                                         scale=svec[:, 1:2])
                    # mn = min(S*acc, 0)
                    nc.vector.tensor_scalar(mn[:, cs], accs[m][:, cs],
                                            svec[:, 0:1], 0.0,
                                            mybir.AluOpType.mult,
                                            mybir.AluOpType.min)
                    # ex = lam*al*exp(mn)
                    nc.scalar.activation(ex[:, cs], mn[:, cs],
                                         mybir.ActivationFunctionType.Exp,
                                         bias=lnb[:])
                    nc.vector.scalar_tensor_tensor(
                        oo[:, cs], ex[:, cs], -lam_al, rl[:, cs],
                        mybir.AluOpType.add, mybir.AluOpType.add)
                nc.sync.dma_start(out_d[m * 128:(m + 1) * 128, :], oo[:])

    nc.compile()
    return nc


def _prepare_in_maps(x, weight, bias, out_idx, in_idx):
    x = np.asarray(x, dtype=np.float32)
    weight = np.asarray(weight, dtype=np.float32)
    bias = np.asarray(bias, dtype=np.float32)
    oi = np.asarray(out_idx).astype(np.int64)
    ii = np.asarray(in_idx).astype(np.int64)

    # densify the edge list; duplicate (i, o) pairs accumulate
    W = np.bincount(ii * OUT_F + oi, weights=weight.astype(np.float64),
                    minlength=IN_F * OUT_F).astype(np.float32)
    W = W.reshape(IN_F, OUT_F)

    dx = CLIP_SIGMA * float(x.std()) / 127.0
    dw = CLIP_SIGMA * float(weight.std()) / 127.0
    if dx == 0.0:
        dx = 1.0
    if dw == 0.0:
        dw = 1.0
    S = dx * dw

    xq = np.clip(np.rint(x / dx), -127, 127).astype(np.int8)   # [B, IN_F]
    Wq = np.clip(np.rint(W / dw), -127, 127).astype(np.int8)   # [IN_F, OUT_F]

    # padded, transposed layouts + bias fold row at k == IN_F
    xtq = np.zeros((KPAD, B), dtype=np.int8)
    xtq[:IN_F] = xq.T
    xtq[IN_F] = np.int8(ONES_ROW_VAL)
    Wpq = np.zeros((KPAD, OUT_F), dtype=np.int8)
    Wpq[:IN_F] = Wq
    bias_q = np.clip(np.rint(bias / (S * ONES_ROW_VAL)), -127, 127)
    Wpq[IN_F] = bias_q.astype(np.int8)

    sc = np.empty((128, 2), dtype=np.float32)
    sc[:, 0] = S
    sc[:, 1] = SELU_SCALE * S

    in_maps = []
    for c in range(NCORES):
        b, o = divmod(c, OS)
        xt_shard = xtq[:, b * BSH:(b + 1) * BSH]           # [KPAD, BSH]
        w_shard = Wpq[:, o * OSH:(o + 1) * OSH]            # [KPAD, OSH]
        wp = np.zeros((KT, 128, WPAD), dtype=np.int8)
        wp[:, :, :OSH] = w_shard.reshape(KT, 128, OSH)
        in_maps.append({
            # partition-major [128, KT, cols]
            "xq": np.ascontiguousarray(
                xt_shard.reshape(KT, 128, BSH).transpose(1, 0, 2)),
            "wq": np.ascontiguousarray(wp.transpose(1, 0, 2)),
            "sc": sc,
        })
    return in_maps


def _assemble(results):
    y = np.empty((B, OUT_F), dtype=np.float32)
    for c in range(NCORES):
        b, o = divmod(c, OS)
        y[b * BSH:(b + 1) * BSH, o * OSH:(o + 1) * OSH] = \
            np.asarray(results[c]["out"]).astype(np.float32)
    return y


def get_compiled():
    global _compiled
    if _compiled is None:
        _compiled = _build()
    return _compiled


def kernel(x, weight, bias, out_idx, in_idx):
    in_maps = _prepare_in_maps(x, weight, bias, out_idx, in_idx)
    nc = get_compiled()
    last_err = None
    for _attempt in range(3):  # retry transient device/runtime hiccups
        try:
            res = run_bass_kernel_spmd(nc, in_maps,
                                       core_ids=list(range(NCORES)))
            return _assemble(res.results)
        except Exception as e:  # noqa: BLE001
            last_err = e
    raise last_err
